# revision 7
# baseline (speedup 1.0000x reference)
"""Trainium2 Bass kernel for the DMAQ/Qatten mixer problem.

Math (per flattened batch sample b, derived from the reference):
  The Qatten attention weights collapse to a constant: softmax over agents of a
  per-head constant logit is uniform 1/A, so w_final[a] = H/A + 1e-10 (same for
  every agent) and the state bias v cancels in aq2 - mq2.  Hence

    v_tot[b]  = sum_a (c * dq[b,a]) * (adv_w[b,a] - 1)
    dq[b,a]   = agent_qs[b,a] - max_q_i[b,a]
    adv_w[b,a]= sum_k kern[k,a] * sigmoid( acts[b,:] @ si_W[k,a,:] + si_b[k,a] )
    kern[k,a] = (|si_keys[k]|+1e-10) * sigmoid(si_agents[k,a])
    c         = w_final (scalar, = H/A + 1e-10)

  attend_mag_regs and head_entropies depend only on (selectors, keys); they are
  computed on host in fp32 exactly as the reference does.

Device pipeline per core (data parallel over the flattened batch, 32768
samples/core on 8 cores):
  - MM1  (PE):  z.T[ka, b] = si_W2[ka, :] @ acts[b, :]   (two K=128 chunks
                accumulated in PSUM; ka = k*16+a, k-major; float32r matmul)
  - ACT:        sig = sigmoid(z.T + si_b[ka])            (per-partition bias)
  - MM2  (PE):  adv[b, a] = sum_ka sig[ka, b] * Kmat[ka, a],
                Kmat[ka, a'] = c*kern[k,a]*delta(a,a')   => adv = c*adv_w
  - DVE TTR:    out[b] = sum_a adv[b,a]*dq[b,a] + (-c * sum_a dq[b,a])
                (single tensor_tensor_reduce per 128-batch tile)

Host-side work is layout packing only (transposed tiling of `actions`,
tile-major packing of agent_qs/max_q_i) plus the tiny parameter folds.
"""

import os
from contextlib import ExitStack

import numpy as np

# -----------------------------------------------------------------------------
# Problem constants (hardcoded per harness contract)
# -----------------------------------------------------------------------------
N_CORES = 8
B0, T = 256, 1024
B_TOTAL = B0 * T              # 262144
B_CORE = B_TOTAL // N_CORES   # 32768
N_AGENTS = 16                 # A
N_HEAD = 4                    # H
EMBED = 64                    # E
N_KERNEL = 4                  # K
ACTION_DIM = N_AGENTS * N_AGENTS  # 256
KA = N_KERNEL * N_AGENTS      # 64
P = 128                       # partitions / batch tile
SUPER = 512                   # batches per MM1 moving operand
G = 4                         # supers per DMA group (2 MiB per acts DMA)
N_SUPER = B_CORE // SUPER     # 64
N_GROUP = N_SUPER // G        # 16
N_TILES = B_CORE // P         # 256

_NC_CACHE: dict = {}


def _build_nc(c_w: float, reps: int = 1, mm_dtype: str = "f32r"):
    """Build the Bass program (identical for all 8 cores — pure SPMD)."""
    import concourse.bacc as bacc
    import concourse.mybir as mybir
    import concourse.tile as tile

    F32 = mybir.dt.float32
    MMDT = {"f32": mybir.dt.float32, "f32r": mybir.dt.float32r,
            "bf16": mybir.dt.bfloat16}[mm_dtype]

    nc = bacc.Bacc(None, target_bir_lowering=False)

    acts_d = nc.declare_dram_parameter(
        "acts", [N_GROUP * P, G * 2 * SUPER], MMDT, isOutput=False
    )
    aqmq_d = nc.declare_dram_parameter(
        "aqmq", [P, 2, N_TILES * N_AGENTS], F32, isOutput=False
    )
    wt_d = nc.declare_dram_parameter("wt", [P, 2, KA], MMDT, isOutput=False)
    km_d = nc.declare_dram_parameter("kmat", [KA, N_AGENTS], F32, isOutput=False)
    sb_d = nc.declare_dram_parameter("sib", [KA, 1], F32, isOutput=False)
    out_d = nc.declare_dram_parameter("out", [P, N_TILES], F32, isOutput=True)

    with ExitStack() as ctx:
        tc = ctx.enter_context(tile.TileContext(nc))
        singles = ctx.enter_context(tc.tile_pool(name="singles", bufs=1))
        acts_pool = ctx.enter_context(tc.tile_pool(name="actsp", bufs=3))
        sig_pool = ctx.enter_context(tc.tile_pool(name="sigp", bufs=3))
        zpool = ctx.enter_context(tc.tile_pool(name="zp", bufs=2, space="PSUM"))
        apool = ctx.enter_context(tc.tile_pool(name="advp", bufs=4, space="PSUM"))
        spool = ctx.enter_context(tc.tile_pool(name="scrp", bufs=4))

        # ---- constant preloads -------------------------------------------------
        wt_t = singles.tile([P, 2, KA], MMDT)
        nc.sync.dma_start(out=wt_t, in_=wt_d[:, :, :])
        km_t = singles.tile([KA, N_AGENTS], F32)
        nc.sync.dma_start(out=km_t, in_=km_d[:, :])
        sib_t = singles.tile([KA, 1], F32)
        nc.sync.dma_start(out=sib_t, in_=sb_d[:, :])

        aqmq_t = singles.tile([P, 2, N_TILES * N_AGENTS], F32)
        nc.sync.dma_start(out=aqmq_t, in_=aqmq_d[:, :, :])

        # dq = aq - mq, once per core; dqs = sum_a dq; cdqsn = -c * dqs
        dq_t = singles.tile([P, N_TILES * N_AGENTS], F32)
        nc.vector.tensor_sub(dq_t, aqmq_t[:, 0, :], aqmq_t[:, 1, :])
        dqs_t = singles.tile([P, N_TILES], F32)
        nc.vector.reduce_sum(
            out=dqs_t,
            in_=dq_t.rearrange("p (t a) -> p t a", a=N_AGENTS),
            axis=mybir.AxisListType.X,
        )
        cdqsn_t = singles.tile([P, N_TILES], F32)
        nc.vector.tensor_scalar_mul(cdqsn_t, dqs_t, -float(c_w))

        out_t = singles.tile([P, N_TILES], F32)

        # ---- streaming main loop ----------------------------------------------
        for _rep in range(reps):
            for g in range(N_GROUP):
                acts_t = acts_pool.tile([P, G * 2 * SUPER], MMDT, name="acts_t")
                nc.sync.dma_start(
                    out=acts_t, in_=acts_d[g * P : (g + 1) * P, :]
                )
                for sp in range(G):
                    s = g * G + sp
                    zt = zpool.tile([KA, SUPER], F32, name="zt")
                    for c in range(2):
                        nc.tensor.matmul(
                            zt,
                            lhsT=wt_t[:, c, :],
                            rhs=acts_t[
                                :, (sp * 2 + c) * SUPER : (sp * 2 + c + 1) * SUPER
                            ],
                            start=(c == 0),
                            stop=(c == 1),
                        )
                    sig_t = sig_pool.tile([KA, SUPER], F32, name="sig_t")
                    nc.scalar.activation(
                        out=sig_t,
                        in_=zt,
                        func=mybir.ActivationFunctionType.Sigmoid,
                        bias=sib_t,
                        scale=1.0,
                    )
                    tps = SUPER // P  # tiles per super = 4
                    adv = apool.tile([P, tps * N_AGENTS], F32, name="adv")
                    for tb in range(tps):
                        nc.tensor.matmul(
                            adv[:, tb * N_AGENTS : (tb + 1) * N_AGENTS],
                            lhsT=sig_t[:, tb * P : (tb + 1) * P],
                            rhs=km_t[:, :],
                            start=True,
                            stop=True,
                        )
                    pr = spool.tile([P, tps * N_AGENTS], F32, name="pr")
                    nc.vector.tensor_mul(
                        pr, adv, dq_t[:, s * tps * N_AGENTS : (s + 1) * tps * N_AGENTS]
                    )
                    rr = spool.tile([P, tps], F32, name="rr")
                    nc.vector.reduce_sum(
                        out=rr,
                        in_=pr.rearrange("p (t a) -> p t a", a=N_AGENTS),
                        axis=mybir.AxisListType.X,
                    )
                    nc.vector.tensor_add(
                        out_t[:, s * tps : (s + 1) * tps],
                        rr,
                        cdqsn_t[:, s * tps : (s + 1) * tps],
                    )

        nc.sync.dma_start(out=out_d[:, :], in_=out_t)

    nc.finalize()
    return nc


def _get_nc(c_w: float, reps: int = 1, mm_dtype: str = "f32r"):
    key = (round(float(c_w), 12), reps, mm_dtype)
    if key not in _NC_CACHE:
        _NC_CACHE[key] = _build_nc(c_w, reps=reps, mm_dtype=mm_dtype)
    return _NC_CACHE[key]


# -----------------------------------------------------------------------------
# Host-side small math (faithful fp32 replication of the reference's tiny ops)
# -----------------------------------------------------------------------------
def _small_outputs(selectors, keys):
    sel = np.asarray(selectors, dtype=np.float32)
    ky = np.asarray(keys, dtype=np.float32)
    H, E = sel.shape
    logits = np.sum(sel * ky, axis=1, dtype=np.float32).astype(np.float32)  # (H,)
    scaled = (logits / np.float32(np.sqrt(np.float32(E)))).astype(np.float32)
    row = np.repeat(scaled[:, None], N_AGENTS, axis=1)  # (H, A) constant rows
    m = row.max(axis=1, keepdims=True)
    e = np.exp((row - m).astype(np.float32)).astype(np.float32)
    aw = (e / e.sum(axis=1, keepdims=True)).astype(np.float32)
    w_final = (aw.sum(axis=0) + np.float32(1e-10)).astype(np.float32)  # (A,)
    regs = (np.float32(0.001) * np.sum(logits * logits, dtype=np.float32)).astype(
        np.float32
    )
    ent = (-np.sum(np.log(aw + np.float32(1e-8)) * aw, axis=1)).astype(np.float32)
    return w_final, regs, ent


def _sigmoid_np(x):
    x = np.asarray(x, dtype=np.float32)
    return (1.0 / (1.0 + np.exp(-x))).astype(np.float32)


# -----------------------------------------------------------------------------
# Input packing (pure layout transforms)
# -----------------------------------------------------------------------------
def _pack_acts(actions):
    """(B0,T,256) fp32 -> per-core (N_GROUP*128, G*2*512), PE-ready transposed
    tiles: row (g*128+dlow), col (sp*1024 + chunk*512 + b) holds
    actions[core*B_CORE + g*G*512 + sp*512 + b, chunk*128 + dlow]."""
    a = np.ascontiguousarray(np.asarray(actions, dtype=np.float32)).reshape(
        N_CORES, N_GROUP, G, SUPER, 2, P
    )
    a = np.ascontiguousarray(a.transpose(0, 1, 5, 2, 4, 3))
    return a.reshape(N_CORES, N_GROUP * P, G * 2 * SUPER)


def _pack_agent(x):
    """(B0,T,16) -> per-core (128, N_TILES*16): row p, col (t*16+a) holds
    x[core*B_CORE + t*128 + p, a]."""
    a = np.ascontiguousarray(np.asarray(x, dtype=np.float32)).reshape(
        N_CORES, N_TILES, P, N_AGENTS
    )
    a = np.ascontiguousarray(a.transpose(0, 2, 1, 3))
    return a.reshape(N_CORES, P, N_TILES * N_AGENTS)


LAST_RESULT = None  # BassKernelResults of the most recent device run


def _run_device(in_maps, c_w, reps=1, mm_dtype="f32r", trace=False):
    global LAST_RESULT
    from concourse.bass_utils import run_bass_kernel_spmd

    nc = _get_nc(c_w, reps=reps, mm_dtype=mm_dtype)
    res = run_bass_kernel_spmd(
        nc, in_maps, core_ids=list(range(N_CORES)), trace=trace
    )
    LAST_RESULT = res
    return res.results


def _prepare(agent_qs, actions, max_q_i, selectors, keys, V, si_keys, si_agents,
             si_W, si_b, mm_dtype="f32r"):
    w_final, regs, ent = _small_outputs(selectors, keys)
    assert float(w_final.max() - w_final.min()) < 1e-12
    c_w = float(w_final[0])

    si_W = np.asarray(si_W, dtype=np.float32)
    si_b = np.asarray(si_b, dtype=np.float32)
    si_keys = np.asarray(si_keys, dtype=np.float32)
    si_agents = np.asarray(si_agents, dtype=np.float32)

    # W2[ka, d] with ka = k*16+a (k-major)  ->  wt[dlow, chunk, ka]
    W2 = si_W.reshape(KA, ACTION_DIM)
    wt_host = np.ascontiguousarray(
        W2.T.reshape(2, P, KA).transpose(1, 0, 2)
    ).astype(np.float32)

    kern = ((np.abs(si_keys)[:, None] + np.float32(1e-10)) * _sigmoid_np(si_agents))
    kern = kern.astype(np.float32)  # (K, A)
    kmat_host = np.zeros((KA, N_AGENTS), dtype=np.float32)
    ka_idx = np.arange(KA)
    kmat_host[ka_idx, ka_idx % N_AGENTS] = (
        w_final[ka_idx % N_AGENTS] * kern.reshape(KA)[ka_idx]
    )

    sib_host = np.ascontiguousarray(si_b.reshape(KA, 1)).astype(np.float32)

    acts_p = _pack_acts(actions)
    if mm_dtype == "bf16":
        import ml_dtypes
        acts_p = acts_p.astype(ml_dtypes.bfloat16)
        wt_host = wt_host.astype(ml_dtypes.bfloat16)
    aq_p = _pack_agent(agent_qs)
    mq_p = _pack_agent(max_q_i)
    aqmq_p = np.ascontiguousarray(np.stack([aq_p, mq_p], axis=1))  # (8, 2, P, cols)
    aqmq_p = aqmq_p.transpose(0, 2, 1, 3)  # (8, P, 2, cols)
    aqmq_p = np.ascontiguousarray(aqmq_p)

    in_maps = [
        {
            "acts": acts_p[ci],
            "aqmq": aqmq_p[ci],
            "wt": wt_host,
            "kmat": kmat_host,
            "sib": sib_host,
        }
        for ci in range(N_CORES)
    ]
    return in_maps, c_w, regs, ent


def _assemble(results):
    parts = [
        np.asarray(results[ci]["out"]).T.reshape(-1) for ci in range(N_CORES)
    ]
    v = np.concatenate(parts).astype(np.float32).reshape(B0, T, 1)
    return v


MM_DTYPE = os.environ.get("KERNEL_MM_DT", "f32r")  # "f32" | "f32r" | "bf16"


def kernel(agent_qs, actions, max_q_i, selectors, keys, V, si_keys, si_agents,
           si_W, si_b):
    in_maps, c_w, regs, ent = _prepare(
        agent_qs, actions, max_q_i, selectors, keys, V, si_keys, si_agents,
        si_W, si_b, mm_dtype=MM_DTYPE
    )
    results = _run_device(in_maps, c_w, reps=1, mm_dtype=MM_DTYPE)
    v_tot = _assemble(results)
    return v_tot, regs, ent
